# revision 1
# baseline (speedup 1.0000x reference)
"""GQA attention block on 8 trn2 NeuronCores.

Sharding: core c = (batch b=c//4, kv-head-pair g=c%4). Each core owns kv heads
{2g, 2g+1} and their 8 query heads (GQA tile mapping: q-head i -> kv-head i%8),
with Wq/Wk/Wv column-sharded and Wo row-sharded; host sums the 4 partial
outputs per batch and adds bo.

Device layout strategy (per core):
  - host stages q^T/k^T/v^T (bf16) so every matmul has its contraction dim on
    partitions with no device-side transposes.
  - RoPE folded into doubled projection weights Wt=[W | rot(W)] (host permuted)
    + elementwise cos/sin combine on DVE.
  - attention computed transposed: scores^T[k,q] = Kp^T(chunk)ᵀ·Qp^T, exp on
    ACT (scale=1/8 folded in, no max subtraction -- scores bounded ~|6|),
    AV via lhsT=Vp with an appended ones column giving the softmax denominator
    for free; normalization via reciprocal + K=1 outer-product broadcast.
  - out^T feeds the final projection as lhsT directly; partial [S,D] fp32 out.
"""

import os
from contextlib import ExitStack

import numpy as np
import ml_dtypes

D = 2048
QH = 32
KVH = 8
HD = 64
B = 2
S = 2048
THETA = 1000000.0
P = 128
NCORES = 8

BF16 = ml_dtypes.bfloat16

_CACHE = {}


def _build_program():
    import concourse.bass as bass
    import concourse.tile as tile
    from concourse import bacc, mybir

    nc = bacc.Bacc(
        "TRN2",
        target_bir_lowering=False,
        debug=False,
        enable_asserts=False,
        num_devices=NCORES,
    )
    bf = mybir.dt.bfloat16
    f32 = mybir.dt.float32

    qT = nc.dram_tensor("qT", [D, S], bf, kind="ExternalInput").ap()
    kT = nc.dram_tensor("kT", [D, S], bf, kind="ExternalInput").ap()
    vT = nc.dram_tensor("vT", [D, S], bf, kind="ExternalInput").ap()
    wqt = nc.dram_tensor("wqt", [D, 1024], bf, kind="ExternalInput").ap()
    wkt = nc.dram_tensor("wkt", [D, 256], bf, kind="ExternalInput").ap()
    wv = nc.dram_tensor("wv", [D, 128], bf, kind="ExternalInput").ap()
    wo = nc.dram_tensor("wo", [512, D], bf, kind="ExternalInput").ap()
    cosr = nc.dram_tensor("cosr", [P, S], f32, kind="ExternalInput").ap()
    sinr = nc.dram_tensor("sinr", [P, S], f32, kind="ExternalInput").ap()
    out = nc.dram_tensor("out", [S, D], f32, kind="ExternalOutput").ap()

    # partitioned DRAM views
    qT3 = qT.rearrange("(o p) s -> p o s", p=P)    # [128, 16, 2048]
    kT3 = kT.rearrange("(o p) s -> p o s", p=P)
    vT3 = vT.rearrange("(o p) s -> p o s", p=P)
    wqt3 = wqt.rearrange("(o p) m -> p o m", p=P)  # [128, 16, 1024]
    wkt3 = wkt.rearrange("(o p) m -> p o m", p=P)  # [128, 16, 256]
    wv3 = wv.rearrange("(o p) m -> p o m", p=P)    # [128, 16, 128]
    wo3 = wo.rearrange("(o p) d -> p o d", p=P)    # [128, 4, 2048]
    out3 = out.rearrange("(t p) d -> p t d", p=P)  # [128, 16, 2048]

    with tile.TileContext(nc) as tc, ExitStack() as ctx:
        const = ctx.enter_context(tc.tile_pool(name="const", bufs=1))
        persist = ctx.enter_context(tc.tile_pool(name="persist", bufs=1))

        # ---- resident weights / tables ----
        wqt_sb = const.tile([P, 16, 1024], bf, tag="wqt")
        nc.sync.dma_start(wqt_sb[:], wqt3[:])
        wkt_sb = const.tile([P, 16, 256], bf, tag="wkt")
        nc.sync.dma_start(wkt_sb[:], wkt3[:])
        wv_sb = const.tile([P, 16, 128], bf, tag="wv")
        nc.sync.dma_start(wv_sb[:], wv3[:])
        wo_sb = const.tile([P, 4, 2048], bf, tag="wo")
        nc.sync.dma_start(wo_sb[:], wo3[:])
        cos_sb = const.tile([P, S], f32, tag="cos")
        nc.sync.dma_start(cos_sb[:], cosr[:])
        sin_sb = const.tile([P, S], f32, tag="sin")
        nc.sync.dma_start(sin_sb[:], sinr[:])
        ones_sb = const.tile([1, 64], f32, tag="ones")
        nc.vector.memset(ones_sb[:], 1.0)

        # ---- persistent intermediates ----
        kpt_b = persist.tile([64, 2, S], bf, tag="kpt")      # rotated K^T per kv head
        qpt_b = persist.tile([64, 8, 2048], bf, tag="qpt")   # rotated Q^T per q head
        vp_sb = persist.tile([P, 16, 130], bf, tag="vp")     # Vp + ones cols
        outT_b = persist.tile([P, 4, 2048], bf, tag="outT")  # unnormalized out^T
        nc.vector.memset(vp_sb[:, :, 64:65], 1.0)
        nc.vector.memset(vp_sb[:, :, 129:130], 1.0)

        # =============== Phase 1-3: projections ===============
        with ExitStack() as pctx:
            bigin = pctx.enter_context(tc.tile_pool(name="bigin", bufs=1))
            kstream = pctx.enter_context(tc.tile_pool(name="kstream", bufs=4))
            ptmp = pctx.enter_context(tc.tile_pool(name="ptmp", bufs=2))
            ppsum = pctx.enter_context(
                tc.tile_pool(name="ppsum", bufs=4, space="PSUM")
            )

            # ---- V projection: direct Vp [s,128] via lhsT = vT slices ----
            for quarter in range(4):
                vh_sb = bigin.tile([P, 16, 512], bf, tag="bigin")
                for o in range(16):
                    nc.sync.dma_start(
                        vh_sb[:, o, :], vT3[:, o, quarter * 512 : (quarter + 1) * 512]
                    )
                for st in range(4):  # s-tiles of 128 within this quarter
                    psv_full = ppsum.tile([P, 512], f32, tag="pp", name="psv")
                    psv = psv_full[:, :128]
                    for o in range(16):
                        nc.tensor.matmul(
                            psv,
                            lhsT=vh_sb[:, o, st * 128 : (st + 1) * 128],
                            rhs=wv_sb[:, o, :],
                            start=(o == 0),
                            stop=(o == 15),
                        )
                    kt_idx = quarter * 4 + st
                    nc.vector.tensor_copy(out=vp_sb[:, kt_idx, 0:64], in_=psv[:, 0:64])
                    nc.vector.tensor_copy(
                        out=vp_sb[:, kt_idx, 65:129], in_=psv[:, 64:128]
                    )

            # ---- K projection + RoPE: KpT_rot per kv head ----
            for ns in range(4):
                ps_kp = ppsum.tile([P, 512], f32, tag="pp")
                ps_kr = ppsum.tile([P, 512], f32, tag="pp")
                for o in range(16):
                    ktile = kstream.tile([P, 512], bf, tag="kt")
                    nc.sync.dma_start(
                        ktile[:], kT3[:, o, ns * 512 : (ns + 1) * 512]
                    )
                    nc.tensor.matmul(
                        ps_kp,
                        lhsT=wkt_sb[:, o, 0:128],
                        rhs=ktile[:],
                        start=(o == 0),
                        stop=(o == 15),
                    )
                    nc.tensor.matmul(
                        ps_kr,
                        lhsT=wkt_sb[:, o, 128:256],
                        rhs=ktile[:],
                        start=(o == 0),
                        stop=(o == 15),
                    )
                sl = slice(ns * 512, (ns + 1) * 512)
                t1 = ptmp.tile([P, 512], f32, tag="t1")
                t2 = ptmp.tile([P, 512], f32, tag="t2")
                nc.vector.tensor_mul(out=t1[:], in0=ps_kp[:], in1=cos_sb[:, sl])
                nc.vector.tensor_mul(out=t2[:], in0=ps_kr[:], in1=sin_sb[:, sl])
                for lh in range(2):
                    lp = slice(lh * 64, lh * 64 + 64)
                    nc.vector.tensor_add(
                        out=kpt_b[:, lh, sl], in0=t1[lp, :], in1=t2[lp, :]
                    )

        # ======= unified pipeline: per s-quarter Qproj -> attn -> outproj =======
        with ExitStack() as mctx:
            bigin = mctx.enter_context(tc.tile_pool(name="bigin2", bufs=1))
            ptmp = mctx.enter_context(tc.tile_pool(name="ptmp2", bufs=2))
            mpsum = mctx.enter_context(
                tc.tile_pool(name="mpsum", bufs=3, space="PSUM")
            )
            apsum = mctx.enter_context(
                tc.tile_pool(name="apsum", bufs=3, space="PSUM")
            )
            opsum = mctx.enter_context(
                tc.tile_pool(name="opsum", bufs=2, space="PSUM")
            )
            epool = mctx.enter_context(tc.tile_pool(name="et", bufs=24))
            ntmp = mctx.enter_context(tc.tile_pool(name="ntmp", bufs=3))
            fout = mctx.enter_context(tc.tile_pool(name="fout", bufs=3))
            scale = 1.0 / float(np.sqrt(HD))
            Exp = mybir.ActivationFunctionType.Exp

            for quarter in range(4):
                # ---- Q projection + RoPE for this s-quarter ----
                qh_sb = bigin.tile([P, 16, 512], bf, tag="bigin")
                for o in range(16):
                    nc.sync.dma_start(
                        qh_sb[:, o, :], qT3[:, o, quarter * 512 : (quarter + 1) * 512]
                    )
                for m in range(4):
                    ps_qp = mpsum.tile([P, 512], f32, tag="pp")
                    for o in range(16):
                        nc.tensor.matmul(
                            ps_qp,
                            lhsT=wqt_sb[:, o, m * 128 : (m + 1) * 128],
                            rhs=qh_sb[:, o, :],
                            start=(o == 0),
                            stop=(o == 15),
                        )
                    gs = slice(quarter * 512, (quarter + 1) * 512)
                    # rotate_half via 32-aligned partition-shifted DVE copies
                    rot = ptmp.tile([P, 512], f32, tag="rot")
                    for hh in range(2):
                        b0 = hh * 64
                        nc.vector.tensor_scalar_mul(
                            rot[b0 : b0 + 32, :], ps_qp[b0 + 32 : b0 + 64, :], -1.0
                        )
                        nc.vector.tensor_copy(
                            out=rot[b0 + 32 : b0 + 64, :], in_=ps_qp[b0 : b0 + 32, :]
                        )
                    t1 = ptmp.tile([P, 512], f32, tag="t1")
                    t2 = ptmp.tile([P, 512], f32, tag="t2")
                    nc.vector.tensor_mul(out=t1[:], in0=ps_qp[:], in1=cos_sb[:, gs])
                    nc.vector.tensor_mul(out=t2[:], in0=rot[:], in1=sin_sb[:, gs])
                    for sub in range(2):
                        lp = slice(sub * 64, sub * 64 + 64)
                        nc.vector.tensor_add(
                            out=qpt_b[:, 2 * m + sub, gs],
                            in0=t1[lp, :],
                            in1=t2[lp, :],
                        )

                # ---- attention for sc = quarter ----
                for lh in range(2):
                    for j in range(4):
                        h = lh * 4 + j
                        hp = slice((h % 2) * 64, (h % 2) * 64 + 64)
                        hc = h // 2
                        ssl = slice(quarter * 512, (quarter + 1) * 512)
                        pso = opsum.tile([65, 512], f32, tag="po")
                        for kt in range(16):
                            pss = apsum.tile([P, 512], f32, tag="ps")
                            nc.tensor.matmul(
                                pss,
                                lhsT=kpt_b[:, lh, kt * 128 : (kt + 1) * 128],
                                rhs=qpt_b[:, h, ssl],
                                start=True,
                                stop=True,
                            )
                            et = epool.tile([P, 512], bf, tag="et", name=f"et{kt}")
                            nc.scalar.activation(
                                out=et[:], in_=pss[:], func=Exp, scale=scale
                            )
                            nc.tensor.matmul(
                                pso,
                                lhsT=vp_sb[:, kt, lh * 65 : (lh + 1) * 65],
                                rhs=et[:],
                                start=(kt == 0),
                                stop=(kt == 15),
                            )
                        recip = ntmp.tile([1, 512], f32, tag="recip")
                        nc.vector.reciprocal(recip[:], pso[64:65, :])
                        bc = ntmp.tile([64, 512], f32, tag="bc")
                        nc.gpsimd.partition_broadcast(bc[:], recip[:])
                        nc.vector.tensor_mul(
                            out=outT_b[hp, hc, ssl], in0=pso[0:64, :], in1=bc[:]
                        )

                # ---- output projection for this quarter's s-tiles ----
                for qi in range(4):
                    qt = quarter * 4 + qi
                    for dn in range(4):
                        psf = mpsum.tile([P, 512], f32, tag="pp", name="psf")
                        for cc in range(4):
                            nc.tensor.matmul(
                                psf,
                                lhsT=outT_b[:, cc, qt * 128 : (qt + 1) * 128],
                                rhs=wo_sb[:, cc, dn * 512 : (dn + 1) * 512],
                                start=(cc == 0),
                                stop=(cc == 3),
                            )
                        of = fout.tile([P, 512], f32, tag="of")
                        nc.any.tensor_copy(out=of[:], in_=psf[:])
                        nc.sync.dma_start(
                            out3[:, qt, dn * 512 : (dn + 1) * 512], of[:]
                        )

    nc.finalize()
    return nc


def _rot_cols(W):
    """(x @ rot_cols(W)) == rotate_half(x @ W), per 64-wide head block."""
    Wr = np.empty_like(W)
    n = W.shape[1] // HD
    for h in range(n):
        blk = W[:, h * HD : (h + 1) * HD]
        Wr[:, h * HD : h * HD + 32] = -blk[:, 32:64]
        Wr[:, h * HD + 32 : h * HD + 64] = blk[:, 0:32]
    return Wr


def _host_inputs(q, k, v, Wq, Wk, Wv, Wo):
    """Build the 8 per-core input dicts."""
    inv_freq = 1.0 / (THETA ** (np.arange(0, HD, 2, dtype=np.float32) / HD))
    t = np.arange(S, dtype=np.float32)
    freqs = np.einsum("i,j->ij", t, inv_freq)
    emb = np.concatenate([freqs, freqs], axis=-1)  # [S, 64]
    cosT = np.ascontiguousarray(np.cos(emb).T, dtype=np.float32)  # [64, S]
    sinT = np.ascontiguousarray(np.sin(emb).T, dtype=np.float32)
    cos_rep = np.concatenate([cosT, cosT], axis=0)  # [128, S]
    sin_rep = np.concatenate([sinT, sinT], axis=0)

    qT = [np.ascontiguousarray(q[b].T).astype(BF16) for b in range(B)]
    kTt = [np.ascontiguousarray(k[b].T).astype(BF16) for b in range(B)]
    vTt = [np.ascontiguousarray(v[b].T).astype(BF16) for b in range(B)]

    in_maps = []
    for c in range(NCORES):
        b, g = divmod(c, 4)
        qheads = [2 * g, 2 * g + 8, 2 * g + 16, 2 * g + 24,
                  2 * g + 1, 2 * g + 9, 2 * g + 17, 2 * g + 25]
        qcols = np.concatenate([np.arange(h * HD, (h + 1) * HD) for h in qheads])
        kvcols = np.arange(2 * g * HD, (2 * g + 2) * HD)

        wq_c = np.ascontiguousarray(Wq[:, qcols])
        wqt_np = np.concatenate([wq_c, _rot_cols(wq_c)], axis=1).astype(BF16)
        wk_c = np.ascontiguousarray(Wk[:, kvcols])
        wkt_np = np.concatenate([wk_c, _rot_cols(wk_c)], axis=1).astype(BF16)
        wv_np = np.ascontiguousarray(Wv[:, kvcols]).astype(BF16)
        wo_np = np.ascontiguousarray(Wo[qcols, :]).astype(BF16)

        in_maps.append({
            "qT": qT[b], "kT": kTt[b], "vT": vTt[b],
            "wqt": wqt_np, "wkt": wkt_np, "wv": wv_np, "wo": wo_np,
            "cosr": cos_rep, "sinr": sin_rep,
        })
    return in_maps


def kernel(q, k, v, attn_mask, Wq, Wk, Wv, Wo, bo):
    from concourse.bass_utils import run_bass_kernel_spmd

    q = np.asarray(q, dtype=np.float32)
    k = np.asarray(k, dtype=np.float32)
    v = np.asarray(v, dtype=np.float32)
    Wq = np.asarray(Wq, dtype=np.float32)
    Wk = np.asarray(Wk, dtype=np.float32)
    Wv = np.asarray(Wv, dtype=np.float32)
    Wo = np.asarray(Wo, dtype=np.float32)
    bo = np.asarray(bo, dtype=np.float32)

    if "nc" not in _CACHE:
        _CACHE["nc"] = _build_program()
    nc = _CACHE["nc"]

    in_maps = _host_inputs(q, k, v, Wq, Wk, Wv, Wo)
    trace = bool(int(os.environ.get("KERNEL_TRACE", "0")))
    res = run_bass_kernel_spmd(nc, in_maps, core_ids=list(range(NCORES)),
                               trace=trace)
    _CACHE["last_result"] = res

    out = np.zeros((B, S, D), dtype=np.float32)
    for c in range(NCORES):
        b = c // 4
        out[b] += np.asarray(res.results[c]["out"], dtype=np.float32)
    out += bo[None, None, :]
    return out



# revision 6
# speedup vs baseline: 1.4098x; 1.4098x over previous
"""GQA attention block on 8 trn2 NeuronCores.

Sharding: core c = (batch b=c//4, kv-head-pair g=c%4). Each core owns kv heads
{2g, 2g+1} and their 8 query heads (GQA tile mapping: q-head i -> kv-head i%8),
with Wq/Wk/Wv column-sharded and Wo row-sharded; host sums the 4 partial
outputs per batch and adds bo.

Device strategy (per core):
  - scores: the two heads of a pair run CONCURRENTLY as 64x128 PE row tiles
    (T0 reads SBUF partitions 0-63 = even head, T8 reads 64-127 = odd head),
    each writing its own PSUM bank of a shared [128,1024] f32 quad (a PSUM
    bank must never be written by two row tiles concurrently).
  - exp on ACT in 1024-wide chunks (amortizes the ~352-cycle ACTIVATE
    overhead), scale=1/8 folded in, bf16 out.
  - AV and all projections run as plain full-array 128x128 matmuls (single
    writer per PSUM bank). AV: lhsT = Vp chunk [128 kpos, 65] with a ones
    column giving the softmax denominator in psum row 64.
  - Q/O projection work is emitted as fine-grained "filler" units between
    attention steps so the PE stays busy while ACT chews exp.
  - RoPE: PSUM evacuated to bf16 SBUF, rotate_half via partition-shifted DVE
    copies, cos/sin combine in bf16 (fast DVE modes).
"""

import os
from contextlib import ExitStack

import numpy as np
import ml_dtypes

D = 2048
QH = 32
KVH = 8
HD = 64
B = 2
S = 2048
THETA = 1000000.0
P = 128
NCORES = 8

BF16 = ml_dtypes.bfloat16

_CACHE = {}


def _build_program():
    import concourse.bass as bass
    import concourse.tile as tile
    from concourse import bacc, mybir

    nc = bacc.Bacc(
        "TRN2",
        target_bir_lowering=False,
        debug=False,
        enable_asserts=False,
        num_devices=NCORES,
    )
    bf = mybir.dt.bfloat16
    f32 = mybir.dt.float32

    qT = nc.dram_tensor("qT", [D, S], bf, kind="ExternalInput").ap()
    kT = nc.dram_tensor("kT", [D, S], bf, kind="ExternalInput").ap()
    vT = nc.dram_tensor("vT", [D, S], bf, kind="ExternalInput").ap()
    wqt = nc.dram_tensor("wqt", [D, 512], bf, kind="ExternalInput").ap()
    wkt = nc.dram_tensor("wkt", [D, 128], bf, kind="ExternalInput").ap()
    wv = nc.dram_tensor("wv", [D, 128], bf, kind="ExternalInput").ap()
    wo = nc.dram_tensor("wo", [512, D], bf, kind="ExternalInput").ap()
    cosr = nc.dram_tensor("cosr", [P, S], bf, kind="ExternalInput").ap()
    sinr = nc.dram_tensor("sinr", [P, S], bf, kind="ExternalInput").ap()
    out = nc.dram_tensor("out", [S, D], f32, kind="ExternalOutput").ap()

    # partitioned DRAM views
    qT3 = qT.rearrange("(o p) s -> p o s", p=P)    # [128, 16, 2048]
    kT3 = kT.rearrange("(o p) s -> p o s", p=P)
    vT3 = vT.rearrange("(o p) s -> p o s", p=P)
    wqt3 = wqt.rearrange("(o p) m -> p o m", p=P)  # [128, 16, 512]
    wkt3 = wkt.rearrange("(o p) m -> p o m", p=P)  # [128, 16, 128]
    wv3 = wv.rearrange("(o p) m -> p o m", p=P)    # [128, 16, 128]
    wo3 = wo.rearrange("(o p) d -> p o d", p=P)    # [128, 4, 2048]
    out3 = out.rearrange("(t p) d -> p t d", p=P)  # [128, 16, 2048]

    scale = 1.0 / float(np.sqrt(HD))
    LO = slice(0, 64)
    HI = slice(64, 128)

    with tile.TileContext(nc) as tc, ExitStack() as ctx:
        Exp = mybir.ActivationFunctionType.Exp
        const = ctx.enter_context(tc.tile_pool(name="const", bufs=1))
        persist = ctx.enter_context(tc.tile_pool(name="persist", bufs=1))
        qpt_pool = ctx.enter_context(tc.tile_pool(name="qptp", bufs=2))
        outT_pool = ctx.enter_context(tc.tile_pool(name="outTp", bufs=2))
        vkin = ctx.enter_context(tc.tile_pool(name="vkin", bufs=2))
        qin = ctx.enter_context(tc.tile_pool(name="qin", bufs=2))
        rtmp = ctx.enter_context(tc.tile_pool(name="rtmp", bufs=2))
        fout = ctx.enter_context(tc.tile_pool(name="fout", bufs=2))
        ntmp = ctx.enter_context(tc.tile_pool(name="ntmp", bufs=2))
        etp = ctx.enter_context(tc.tile_pool(name="etp", bufs=3))
        qpsum = ctx.enter_context(tc.tile_pool(name="qpsum", bufs=2, space="PSUM"))
        apsum = ctx.enter_context(tc.tile_pool(name="apsum", bufs=2, space="PSUM"))
        ppsum = ctx.enter_context(tc.tile_pool(name="ppsum", bufs=2, space="PSUM"))

        # ---- resident weights / tables ----
        wqt_sb = const.tile([P, 16, 512], bf, tag="wqt")
        nc.sync.dma_start(wqt_sb[:], wqt3[:])
        wkt_sb = const.tile([P, 16, 128], bf, tag="wkt")
        nc.sync.dma_start(wkt_sb[:], wkt3[:])
        wv_sb = const.tile([P, 16, 128], bf, tag="wv")
        nc.sync.dma_start(wv_sb[:], wv3[:])
        wo_sb = const.tile([P, 4, 2048], bf, tag="wo")
        nc.sync.dma_start(wo_sb[:], wo3[:])
        cos_sb = const.tile([P, S], bf, tag="cos")
        nc.sync.dma_start(cos_sb[:], cosr[:])
        sin_sb = const.tile([P, S], bf, tag="sin")
        nc.sync.dma_start(sin_sb[:], sinr[:])

        # ---- persistent intermediates ----
        kpt_b = persist.tile([P, S], bf, tag="kpt")       # rotated K^T pair-stacked
        vp_sb = persist.tile([P, 16, 130], bf, tag="vp")  # Vp + ones cols
        nc.vector.memset(vp_sb[:, :, 64:65], 1.0)
        nc.vector.memset(vp_sb[:, :, 129:130], 1.0)

        def rope(ps, gs, dst):
            """RoPE: ps [128,512] f32 psum (pair-stacked head dims) ->
            dst bf16 [128,512]. Evacuate early to free the bank, then bf16
            DVE ops (rotate_half = partition-shifted copies)."""
            ev = rtmp.tile([P, 512], bf, tag="ev", name="ev")
            nc.vector.tensor_copy(out=ev[:], in_=ps[:])
            rot = rtmp.tile([P, 512], bf, tag="rot", name="rot")
            for b0 in (0, 64):
                nc.vector.tensor_scalar_mul(
                    rot[b0 : b0 + 32, :], ev[b0 + 32 : b0 + 64, :], -1.0
                )
                nc.vector.tensor_copy(
                    out=rot[b0 + 32 : b0 + 64, :], in_=ev[b0 : b0 + 32, :]
                )
            t1 = rtmp.tile([P, 512], bf, tag="t1", name="t1")
            t2 = rtmp.tile([P, 512], bf, tag="t2", name="t2")
            nc.vector.tensor_mul(out=t1[:], in0=ev[:], in1=cos_sb[:, gs])
            nc.vector.tensor_mul(out=t2[:], in0=rot[:], in1=sin_sb[:, gs])
            nc.vector.tensor_add(out=dst, in0=t1[:], in1=t2[:])

        def accum(chunks, nacc, get_lhsT, get_rhs, alloc, consume,
                  prep=None, unit=2):
            """Generator emitting full-array PSUM accumulation chains, one
            chunk per bank (single writer). Yields every `unit` matmuls."""
            cnt = 0
            for desc in list(chunks):
                if prep is not None:
                    prep(desc)
                ps = alloc(desc)
                for o in range(nacc):
                    nc.tensor.matmul(
                        ps, lhsT=get_lhsT(desc, o), rhs=get_rhs(desc, o),
                        start=(o == 0), stop=(o == nacc - 1),
                    )
                    cnt += 1
                    if cnt >= unit:
                        cnt = 0
                        yield
                consume(ps, desc)

        def run_all(gen):
            for _ in gen:
                pass

        # ================= prologue: V projection =================
        # vp[s,128] per s-tile via stationary vT s-tiles, moving wv.
        def v_phase():
            vh_tiles = {}

            def prep(st):
                sc = st // 4
                if st % 4 == 0:
                    vh = vkin.tile([P, 16, 512], bf, tag="vkin", name="vh")
                    for o in range(16):
                        nc.sync.dma_start(
                            vh[:, o, :], vT3[:, o, sc * 512 : (sc + 1) * 512]
                        )
                    vh_tiles[sc] = vh

            def alloc(st):
                return qpsum.tile([P, 1024], f32, tag="qp", name="psv")[:, :128]

            def get_lhsT(st, o):
                return vh_tiles[st // 4][:, o, (st % 4) * 128 : (st % 4 + 1) * 128]

            def get_rhs(st, o):
                return wv_sb[:, o, :]

            def consume(ps, st):
                nc.vector.tensor_copy(out=vp_sb[:, st, 0:64], in_=ps[:, 0:64])
                nc.vector.tensor_copy(out=vp_sb[:, st, 65:129], in_=ps[:, 64:128])

            run_all(accum(range(16), 16, get_lhsT, get_rhs, alloc, consume,
                          prep=prep))

        v_phase()

        # ================= prologue: K projection + RoPE =================
        def k_phase():
            kh_tiles = {}

            def prep(ns):
                kh = vkin.tile([P, 16, 512], bf, tag="vkin", name="kh")
                for o in range(16):
                    nc.sync.dma_start(
                        kh[:, o, :], kT3[:, o, ns * 512 : (ns + 1) * 512]
                    )
                kh_tiles[ns] = kh

            def alloc(ns):
                return ppsum.tile([P, 512], f32, tag="pp", name="psk")

            def get_lhsT(ns, o):
                return wkt_sb[:, o, :]

            def get_rhs(ns, o):
                return kh_tiles[ns][:, o, :]

            def consume(ps, ns):
                gs = slice(ns * 512, (ns + 1) * 512)
                rope(ps, gs, kpt_b[:, gs])

            run_all(accum(range(4), 16, get_lhsT, get_rhs, alloc, consume,
                          prep=prep))

        k_phase()

        # ================= Q projection (one quarter) =================
        qpt_tiles = {}

        def load_qh(quarter):
            qh_sb = qin.tile([P, 16, 512], bf, tag="qin", name="qh")
            for o in range(16):
                nc.sync.dma_start(
                    qh_sb[:, o, :], qT3[:, o, quarter * 512 : (quarter + 1) * 512]
                )
            return qh_sb

        def qproj_gen(quarter, qh_sb):
            gs = slice(quarter * 512, (quarter + 1) * 512)
            qpt_tiles[quarter] = qpt_pool.tile([P, 4, 512], bf, tag="qpt", name="qpt_q")

            def alloc(m):
                return ppsum.tile([P, 512], f32, tag="pp", name="psq")

            def get_lhsT(m, o):
                return wqt_sb[:, o, m * 128 : (m + 1) * 128]

            def get_rhs(m, o):
                return qh_sb[:, o, :]

            def consume(ps, m):
                rope(ps, gs, qpt_tiles[quarter][:, m, :])

            return accum(range(4), 16, get_lhsT, get_rhs, alloc, consume)

        # ================= output projection (one quarter) =================
        outT_tiles = {}

        def outproj_gen(quarter):
            combos = [(qi, dn) for qi in range(4) for dn in range(4)]
            outT_q = outT_tiles[quarter]

            def alloc(c):
                return ppsum.tile([P, 512], f32, tag="pp", name="psf")

            def get_lhsT(c, o):
                qi, dn = c
                return outT_q[:, o, qi * 128 : (qi + 1) * 128]

            def get_rhs(c, o):
                qi, dn = c
                return wo_sb[:, o, dn * 512 : (dn + 1) * 512]

            def consume(ps, c):
                qi, dn = c
                of = fout.tile([P, 512], f32, tag="of", name="of")
                nc.vector.tensor_copy(out=of[:], in_=ps[:])
                nc.sync.dma_start(
                    out3[:, quarter * 4 + qi, dn * 512 : (dn + 1) * 512], of[:]
                )

            return accum(combos, 4, get_lhsT, get_rhs, alloc, consume)

        # ---- quarter 0 Q projection up front ----
        qh0 = load_qh(0)
        run_all(qproj_gen(0, qh0))

        # ================= main loop: attention + filler =================
        for quarter in range(4):
            qpt_q = qpt_tiles[quarter]
            outT_q = outT_pool.tile([P, 4, 512], bf, tag="outT", name="outT_q")
            outT_tiles[quarter] = outT_q

            filler = []
            if quarter < 3:
                qh_next = load_qh(quarter + 1)
                filler.append(qproj_gen(quarter + 1, qh_next))
            if quarter > 0:
                filler.append(outproj_gen(quarter - 1))

            def pop_filler():
                while filler:
                    try:
                        next(filler[0])
                        return
                    except StopIteration:
                        filler.pop(0)

            for pr in range(4):  # head pair
                av0 = apsum.tile([65, 512], f32, tag="av", name="av0")
                av1 = apsum.tile([65, 512], f32, tag="av", name="av1")

                def av_step(pet, pkt, last):
                    first = pkt == 0
                    nc.tensor.matmul(
                        av0, lhsT=vp_sb[:, pkt, 0:65], rhs=pet[:, 0:512],
                        start=first, stop=last,
                    )
                    nc.tensor.matmul(
                        av1, lhsT=vp_sb[:, pkt, 65:130], rhs=pet[:, 512:1024],
                        start=first, stop=last,
                    )

                prev_et = None
                for kt in range(16):
                    quad = qpsum.tile([P, 1024], f32, tag="qp", name="quad")
                    ksl = slice(kt * 128, (kt + 1) * 128)
                    # paired scores: T0 = even head, T8 = odd head
                    nc.tensor.matmul(
                        quad[:, 0:512], lhsT=kpt_b[LO, ksl],
                        rhs=qpt_q[LO, pr, :], start=True, stop=True,
                    )
                    nc.tensor.matmul(
                        quad[:, 512:1024], lhsT=kpt_b[HI, ksl],
                        rhs=qpt_q[HI, pr, :], start=True, stop=True,
                    )
                    et = etp.tile([P, 1024], bf, tag="et", name="et")
                    nc.scalar.activation(
                        out=et[:], in_=quad[:], func=Exp, scale=scale
                    )
                    pop_filler()
                    if prev_et is not None:
                        av_step(*prev_et, last=False)
                    prev_et = (et, kt)
                av_step(*prev_et, last=True)

                # ---- normalization ----
                for e, av in ((0, av0), (1, av1)):
                    recip = ntmp.tile([1, 512], f32, tag="recip", name="recip")
                    nc.vector.reciprocal(recip[:], av[64:65, :])
                    bc = ntmp.tile([64, 512], f32, tag="bc", name="bc")
                    nc.gpsimd.partition_broadcast(bc[:], recip[:])
                    hp = slice(e * 64, e * 64 + 64)
                    nc.vector.tensor_mul(
                        out=outT_q[hp, pr, :], in0=av[0:64, :], in1=bc[:]
                    )

            # drain remaining filler for this quarter
            while filler:
                try:
                    next(filler[0])
                except StopIteration:
                    filler.pop(0)

        # epilogue: out projection of last quarter
        run_all(outproj_gen(3))

    nc.finalize()
    return nc


def _host_inputs(q, k, v, Wq, Wk, Wv, Wo):
    """Build the 8 per-core input dicts."""
    inv_freq = 1.0 / (THETA ** (np.arange(0, HD, 2, dtype=np.float32) / HD))
    t = np.arange(S, dtype=np.float32)
    freqs = np.einsum("i,j->ij", t, inv_freq)
    emb = np.concatenate([freqs, freqs], axis=-1)  # [S, 64]
    cosT = np.ascontiguousarray(np.cos(emb).T, dtype=np.float32)  # [64, S]
    sinT = np.ascontiguousarray(np.sin(emb).T, dtype=np.float32)
    cos_rep = np.concatenate([cosT, cosT], axis=0).astype(BF16)  # [128, S]
    sin_rep = np.concatenate([sinT, sinT], axis=0).astype(BF16)

    qT = [np.ascontiguousarray(q[b].T).astype(BF16) for b in range(B)]
    kTt = [np.ascontiguousarray(k[b].T).astype(BF16) for b in range(B)]
    vTt = [np.ascontiguousarray(v[b].T).astype(BF16) for b in range(B)]

    in_maps = []
    for c in range(NCORES):
        b, g = divmod(c, 4)
        # pair-interleaved: chunk i of 128 cols = (kv0 q-head i, kv1 q-head i)
        qheads = [2 * g, 2 * g + 1, 2 * g + 8, 2 * g + 9,
                  2 * g + 16, 2 * g + 17, 2 * g + 24, 2 * g + 25]
        qcols = np.concatenate([np.arange(h * HD, (h + 1) * HD) for h in qheads])
        kvcols = np.arange(2 * g * HD, (2 * g + 2) * HD)

        wqt_np = np.ascontiguousarray(Wq[:, qcols]).astype(BF16)
        wkt_np = np.ascontiguousarray(Wk[:, kvcols]).astype(BF16)
        wv_np = np.ascontiguousarray(Wv[:, kvcols]).astype(BF16)
        wo_np = np.ascontiguousarray(Wo[qcols, :]).astype(BF16)

        in_maps.append({
            "qT": qT[b], "kT": kTt[b], "vT": vTt[b],
            "wqt": wqt_np, "wkt": wkt_np, "wv": wv_np, "wo": wo_np,
            "cosr": cos_rep, "sinr": sin_rep,
        })
    return in_maps


def kernel(q, k, v, attn_mask, Wq, Wk, Wv, Wo, bo):
    from concourse.bass_utils import run_bass_kernel_spmd

    q = np.asarray(q, dtype=np.float32)
    k = np.asarray(k, dtype=np.float32)
    v = np.asarray(v, dtype=np.float32)
    Wq = np.asarray(Wq, dtype=np.float32)
    Wk = np.asarray(Wk, dtype=np.float32)
    Wv = np.asarray(Wv, dtype=np.float32)
    Wo = np.asarray(Wo, dtype=np.float32)
    bo = np.asarray(bo, dtype=np.float32)

    if "nc" not in _CACHE:
        _CACHE["nc"] = _build_program()
    nc = _CACHE["nc"]

    in_maps = _host_inputs(q, k, v, Wq, Wk, Wv, Wo)
    trace = bool(int(os.environ.get("KERNEL_TRACE", "0")))
    res = run_bass_kernel_spmd(nc, in_maps, core_ids=list(range(NCORES)),
                               trace=trace)
    _CACHE["last_result"] = res

    out = np.zeros((B, S, D), dtype=np.float32)
    for c in range(NCORES):
        b = c // 4
        out[b] += np.asarray(res.results[c]["out"], dtype=np.float32)
    out += bo[None, None, :]
    return out


# revision 8
# speedup vs baseline: 1.6523x; 1.1721x over previous
"""GQA attention block on 8 trn2 NeuronCores.

Sharding: core c = (batch b=c//4, kv-head-pair g=c%4). Each core owns kv heads
{2g, 2g+1} and their 8 query heads (GQA tile mapping: q-head i -> kv-head i%8),
with Wq/Wk/Wv column-sharded and Wo row-sharded; host sums the 4 partial
outputs per batch and adds bo.

Device strategy (per core):
  - scores: the two heads of a pair run CONCURRENTLY as 64x128 PE row tiles
    (T0 reads SBUF partitions 0-63 = even head, T8 reads 64-127 = odd head),
    each writing its own PSUM bank of a shared [128,1024] f32 quad (a PSUM
    bank must never be written by two row tiles concurrently).
  - exp on ACT in 1024-wide chunks (amortizes the ~352-cycle ACTIVATE
    overhead), scale=1/8 folded in, bf16 out.
  - AV and all projections run as plain full-array 128x128 matmuls (single
    writer per PSUM bank). AV: lhsT = Vp chunk [128 kpos, 65] with a ones
    column giving the softmax denominator in psum row 64.
  - Q/O projection work is emitted as fine-grained "filler" units between
    attention steps so the PE stays busy while ACT chews exp.
  - RoPE: PSUM evacuated to bf16 SBUF, rotate_half via partition-shifted DVE
    copies, cos/sin combine in bf16 (fast DVE modes).
"""

import os
from contextlib import ExitStack

import numpy as np
import ml_dtypes

D = 2048
QH = 32
KVH = 8
HD = 64
B = 2
S = 2048
THETA = 1000000.0
P = 128
NCORES = 8

BF16 = ml_dtypes.bfloat16

_CACHE = {}


def _build_program():
    import concourse.bass as bass
    import concourse.tile as tile
    from concourse import bacc, mybir

    nc = bacc.Bacc(
        "TRN2",
        target_bir_lowering=False,
        debug=False,
        enable_asserts=False,
        num_devices=NCORES,
    )
    bf = mybir.dt.bfloat16
    f32 = mybir.dt.float32

    qT = nc.dram_tensor("qT", [D, S], bf, kind="ExternalInput").ap()
    kT = nc.dram_tensor("kT", [D, S], bf, kind="ExternalInput").ap()
    vT = nc.dram_tensor("vT", [D, S], bf, kind="ExternalInput").ap()
    wqt = nc.dram_tensor("wqt", [D, 512], bf, kind="ExternalInput").ap()
    wkt = nc.dram_tensor("wkt", [D, 128], bf, kind="ExternalInput").ap()
    wv = nc.dram_tensor("wv", [D, 128], bf, kind="ExternalInput").ap()
    wo = nc.dram_tensor("wo", [512, D], bf, kind="ExternalInput").ap()
    cosr = nc.dram_tensor("cosr", [P, S], bf, kind="ExternalInput").ap()
    sinr = nc.dram_tensor("sinr", [P, S], bf, kind="ExternalInput").ap()
    out = nc.dram_tensor("out", [S, D], f32, kind="ExternalOutput").ap()

    # partitioned DRAM views
    qT3 = qT.rearrange("(o p) s -> p o s", p=P)    # [128, 16, 2048]
    kT3 = kT.rearrange("(o p) s -> p o s", p=P)
    vT3 = vT.rearrange("(o p) s -> p o s", p=P)
    wqt3 = wqt.rearrange("(o p) m -> p o m", p=P)  # [128, 16, 512]
    wkt3 = wkt.rearrange("(o p) m -> p o m", p=P)  # [128, 16, 128]
    wv3 = wv.rearrange("(o p) m -> p o m", p=P)    # [128, 16, 128]
    wo3 = wo.rearrange("(o p) d -> p o d", p=P)    # [128, 4, 2048]
    out3 = out.rearrange("(t p) d -> p t d", p=P)  # [128, 16, 2048]

    scale = 1.0 / float(np.sqrt(HD))
    LO = slice(0, 64)
    HI = slice(64, 128)

    with tile.TileContext(nc) as tc, ExitStack() as ctx:
        Exp = mybir.ActivationFunctionType.Exp
        const = ctx.enter_context(tc.tile_pool(name="const", bufs=1))
        persist = ctx.enter_context(tc.tile_pool(name="persist", bufs=1))
        qpt_pool = ctx.enter_context(tc.tile_pool(name="qptp", bufs=2))
        outT_pool = ctx.enter_context(tc.tile_pool(name="outTp", bufs=2))
        vkin = ctx.enter_context(tc.tile_pool(name="vkin", bufs=2))
        qin = ctx.enter_context(tc.tile_pool(name="qin", bufs=2))
        rtmp = ctx.enter_context(tc.tile_pool(name="rtmp", bufs=2))
        fout = ctx.enter_context(tc.tile_pool(name="fout", bufs=2))
        ntmp = ctx.enter_context(tc.tile_pool(name="ntmp", bufs=2))
        etp = ctx.enter_context(tc.tile_pool(name="etp", bufs=3))
        qpsum = ctx.enter_context(tc.tile_pool(name="qpsum", bufs=2, space="PSUM"))
        apsum = ctx.enter_context(tc.tile_pool(name="apsum", bufs=2, space="PSUM"))
        ppsum = ctx.enter_context(tc.tile_pool(name="ppsum", bufs=2, space="PSUM"))

        # ---- resident weights / tables (small V/K weights first so the
        # V projection can start while the big tables stream in) ----
        wv_sb = const.tile([P, 16, 128], bf, tag="wv")
        nc.sync.dma_start(wv_sb[:], wv3[:])
        wkt_sb = const.tile([P, 16, 128], bf, tag="wkt")
        nc.sync.dma_start(wkt_sb[:], wkt3[:])
        cos_sb = const.tile([P, S], bf, tag="cos")
        nc.sync.dma_start(cos_sb[:], cosr[:])
        sin_sb = const.tile([P, S], bf, tag="sin")
        nc.sync.dma_start(sin_sb[:], sinr[:])
        wqt_sb = const.tile([P, 16, 512], bf, tag="wqt")
        nc.sync.dma_start(wqt_sb[:], wqt3[:])
        wo_sb = const.tile([P, 4, 2048], bf, tag="wo")
        nc.sync.dma_start(wo_sb[:], wo3[:])

        # ---- persistent intermediates ----
        kpt_b = persist.tile([P, S], bf, tag="kpt")       # rotated K^T pair-stacked
        vp_sb = persist.tile([P, 16, 130], bf, tag="vp")  # Vp + ones cols
        nc.vector.memset(vp_sb[:, :, 64:65], 1.0)
        nc.vector.memset(vp_sb[:, :, 129:130], 1.0)

        def rope(ps, gs, dst):
            """RoPE: ps [128,512] f32 psum (pair-stacked head dims) ->
            dst bf16 [128,512]. Evacuate early to free the bank, then bf16
            DVE ops (rotate_half = partition-shifted copies)."""
            ev = rtmp.tile([P, 512], bf, tag="ev", name="ev")
            nc.vector.tensor_copy(out=ev[:], in_=ps[:])
            rot = rtmp.tile([P, 512], bf, tag="rot", name="rot")
            for b0 in (0, 64):
                nc.vector.tensor_scalar_mul(
                    rot[b0 : b0 + 32, :], ev[b0 + 32 : b0 + 64, :], -1.0
                )
                nc.vector.tensor_copy(
                    out=rot[b0 + 32 : b0 + 64, :], in_=ev[b0 : b0 + 32, :]
                )
            t1 = rtmp.tile([P, 512], bf, tag="t1", name="t1")
            t2 = rtmp.tile([P, 512], bf, tag="t2", name="t2")
            nc.vector.tensor_mul(out=t1[:], in0=ev[:], in1=cos_sb[:, gs])
            nc.vector.tensor_mul(out=t2[:], in0=rot[:], in1=sin_sb[:, gs])
            nc.vector.tensor_add(out=dst, in0=t1[:], in1=t2[:])

        def accum(chunks, nacc, get_lhsT, get_rhs, alloc, consume,
                  prep=None, unit=2):
            """Generator emitting full-array PSUM accumulation chains, one
            chunk per bank (single writer). Yields every `unit` matmuls."""
            cnt = 0
            for desc in list(chunks):
                if prep is not None:
                    prep(desc)
                ps = alloc(desc)
                for o in range(nacc):
                    nc.tensor.matmul(
                        ps, lhsT=get_lhsT(desc, o), rhs=get_rhs(desc, o),
                        start=(o == 0), stop=(o == nacc - 1),
                    )
                    cnt += 1
                    if cnt >= unit:
                        cnt = 0
                        yield
                consume(ps, desc)

        def run_all(gen):
            for _ in gen:
                pass

        # ================= prologue: V projection =================
        # vp[s,128] per s-tile via stationary vT s-tiles, moving wv.
        def v_phase():
            vh_tiles = {}

            def prep(st):
                sc = st // 4
                if st % 4 == 0 and sc not in vh_tiles:
                    vh = vkin.tile([P, 16, 512], bf, tag="vkin", name="vh")
                    for o4 in range(0, 16, 4):
                        nc.sync.dma_start(
                            vh[:, o4 : o4 + 4, :],
                            vT3[:, o4 : o4 + 4, sc * 512 : (sc + 1) * 512],
                        )
                    vh_tiles[sc] = vh

            def alloc(st):
                return qpsum.tile([P, 1024], f32, tag="qp", name="psv")[:, :128]

            def get_lhsT(st, o):
                return vh_tiles[st // 4][:, o, (st % 4) * 128 : (st % 4 + 1) * 128]

            def get_rhs(st, o):
                return wv_sb[:, o, :]

            def consume(ps, st):
                nc.vector.tensor_copy(out=vp_sb[:, st, 0:64], in_=ps[:, 0:64])
                nc.vector.tensor_copy(out=vp_sb[:, st, 65:129], in_=ps[:, 64:128])

            run_all(accum(range(16), 16, get_lhsT, get_rhs, alloc, consume,
                          prep=prep))

        # prefetch the first K chunk and quarter-0 Q input alongside V
        kh_prefetch = {}
        kh0 = vkin.tile([P, 16, 512], bf, tag="kh0", name="kh0", bufs=1)
        for o4 in range(0, 16, 4):
            nc.sync.dma_start(kh0[:, o4 : o4 + 4, :], kT3[:, o4 : o4 + 4, 0:512])
        kh_prefetch[0] = kh0

        v_phase()

        # ================= prologue: K projection + RoPE =================
        def k_phase():
            kh_tiles = dict(kh_prefetch)

            def prep(ns):
                if ns in kh_tiles:
                    return
                kh = vkin.tile([P, 16, 512], bf, tag="vkin", name="kh")
                for o4 in range(0, 16, 4):
                    nc.sync.dma_start(
                        kh[:, o4 : o4 + 4, :],
                        kT3[:, o4 : o4 + 4, ns * 512 : (ns + 1) * 512],
                    )
                kh_tiles[ns] = kh

            def alloc(ns):
                return ppsum.tile([P, 512], f32, tag="pp", name="psk")

            def get_lhsT(ns, o):
                return wkt_sb[:, o, :]

            def get_rhs(ns, o):
                return kh_tiles[ns][:, o, :]

            def consume(ps, ns):
                gs = slice(ns * 512, (ns + 1) * 512)
                rope(ps, gs, kpt_b[:, gs])

            run_all(accum(range(4), 16, get_lhsT, get_rhs, alloc, consume,
                          prep=prep))

        k_phase()

        # ================= Q projection (one quarter) =================
        qpt_tiles = {}

        def load_qh(quarter):
            qh_sb = qin.tile([P, 16, 512], bf, tag="qin", name="qh")
            for o4 in range(0, 16, 4):
                nc.sync.dma_start(
                    qh_sb[:, o4 : o4 + 4, :],
                    qT3[:, o4 : o4 + 4, quarter * 512 : (quarter + 1) * 512],
                )
            return qh_sb

        def qproj_gen(quarter, qh_sb):
            gs = slice(quarter * 512, (quarter + 1) * 512)
            qpt_tiles[quarter] = qpt_pool.tile([P, 4, 512], bf, tag="qpt", name="qpt_q")

            def alloc(m):
                return ppsum.tile([P, 512], f32, tag="pp", name="psq")

            def get_lhsT(m, o):
                return wqt_sb[:, o, m * 128 : (m + 1) * 128]

            def get_rhs(m, o):
                return qh_sb[:, o, :]

            def consume(ps, m):
                rope(ps, gs, qpt_tiles[quarter][:, m, :])

            return accum(range(4), 16, get_lhsT, get_rhs, alloc, consume)

        # ================= output projection (one quarter) =================
        outT_tiles = {}

        def outproj_gen(quarter):
            combos = [(qi, dn) for qi in range(4) for dn in range(4)]
            outT_q = outT_tiles[quarter]

            def alloc(c):
                return ppsum.tile([P, 512], f32, tag="pp", name="psf")

            def get_lhsT(c, o):
                qi, dn = c
                return outT_q[:, o, qi * 128 : (qi + 1) * 128]

            def get_rhs(c, o):
                qi, dn = c
                return wo_sb[:, o, dn * 512 : (dn + 1) * 512]

            def consume(ps, c):
                qi, dn = c
                of = fout.tile([P, 512], f32, tag="of", name="of")
                nc.vector.tensor_copy(out=of[:], in_=ps[:])
                nc.sync.dma_start(
                    out3[:, quarter * 4 + qi, dn * 512 : (dn + 1) * 512], of[:]
                )

            return accum(combos, 4, get_lhsT, get_rhs, alloc, consume)

        # ---- quarter 0 Q projection up front ----
        qh0 = load_qh(0)
        run_all(qproj_gen(0, qh0))

        # ================= main loop: attention + filler =================
        for quarter in range(4):
            qpt_q = qpt_tiles[quarter]
            outT_q = outT_pool.tile([P, 4, 512], bf, tag="outT", name="outT_q")
            outT_tiles[quarter] = outT_q

            filler = []
            if quarter < 3:
                qh_next = load_qh(quarter + 1)
                filler.append(qproj_gen(quarter + 1, qh_next))
            if quarter > 0:
                filler.append(outproj_gen(quarter - 1))

            def pop_filler():
                while filler:
                    try:
                        next(filler[0])
                        return
                    except StopIteration:
                        filler.pop(0)

            for pr in range(4):  # head pair
                av0 = apsum.tile([65, 512], f32, tag="av", name="av0")
                av1 = apsum.tile([65, 512], f32, tag="av", name="av1")

                def av_step(pet, pkt, last):
                    first = pkt == 0
                    nc.tensor.matmul(
                        av0, lhsT=vp_sb[:, pkt, 0:65], rhs=pet[:, 0:512],
                        start=first, stop=last,
                    )
                    nc.tensor.matmul(
                        av1, lhsT=vp_sb[:, pkt, 65:130], rhs=pet[:, 512:1024],
                        start=first, stop=last,
                    )

                prev_et = None
                for kt in range(16):
                    quad = qpsum.tile([P, 1024], f32, tag="qp", name="quad")
                    ksl = slice(kt * 128, (kt + 1) * 128)
                    # paired scores: T0 = even head, T8 = odd head
                    nc.tensor.matmul(
                        quad[:, 0:512], lhsT=kpt_b[LO, ksl],
                        rhs=qpt_q[LO, pr, :], start=True, stop=True,
                    )
                    nc.tensor.matmul(
                        quad[:, 512:1024], lhsT=kpt_b[HI, ksl],
                        rhs=qpt_q[HI, pr, :], start=True, stop=True,
                    )
                    et = etp.tile([P, 1024], bf, tag="et", name="et")
                    nc.scalar.activation(
                        out=et[:], in_=quad[:], func=Exp, scale=scale
                    )
                    pop_filler()
                    if prev_et is not None:
                        av_step(*prev_et, last=False)
                    prev_et = (et, kt)
                av_step(*prev_et, last=True)

                # ---- normalization (evacuate psum first: frees the AV
                # bank in ~0.7us so the next pair's AV is not blocked on the
                # slow reciprocal chain) ----
                for e, av in ((0, av0), (1, av1)):
                    avc = ntmp.tile([65, 512], f32, tag="avc", name="avc")
                    nc.vector.tensor_copy(out=avc[:], in_=av[:])
                    recip = ntmp.tile([1, 512], f32, tag="recip", name="recip")
                    nc.vector.reciprocal(recip[:], avc[64:65, :])
                    bc = ntmp.tile([64, 512], f32, tag="bc", name="bc")
                    nc.gpsimd.partition_broadcast(bc[:], recip[:])
                    hp = slice(e * 64, e * 64 + 64)
                    nc.vector.tensor_mul(
                        out=outT_q[hp, pr, :], in0=avc[0:64, :], in1=bc[:]
                    )

            # drain remaining filler for this quarter
            while filler:
                try:
                    next(filler[0])
                except StopIteration:
                    filler.pop(0)

        # epilogue: out projection of last quarter
        run_all(outproj_gen(3))

    nc.finalize()
    return nc


def _host_inputs(q, k, v, Wq, Wk, Wv, Wo):
    """Build the 8 per-core input dicts."""
    inv_freq = 1.0 / (THETA ** (np.arange(0, HD, 2, dtype=np.float32) / HD))
    t = np.arange(S, dtype=np.float32)
    freqs = np.einsum("i,j->ij", t, inv_freq)
    emb = np.concatenate([freqs, freqs], axis=-1)  # [S, 64]
    cosT = np.ascontiguousarray(np.cos(emb).T, dtype=np.float32)  # [64, S]
    sinT = np.ascontiguousarray(np.sin(emb).T, dtype=np.float32)
    cos_rep = np.concatenate([cosT, cosT], axis=0).astype(BF16)  # [128, S]
    sin_rep = np.concatenate([sinT, sinT], axis=0).astype(BF16)

    qT = [np.ascontiguousarray(q[b].T).astype(BF16) for b in range(B)]
    kTt = [np.ascontiguousarray(k[b].T).astype(BF16) for b in range(B)]
    vTt = [np.ascontiguousarray(v[b].T).astype(BF16) for b in range(B)]

    in_maps = []
    for c in range(NCORES):
        b, g = divmod(c, 4)
        # pair-interleaved: chunk i of 128 cols = (kv0 q-head i, kv1 q-head i)
        qheads = [2 * g, 2 * g + 1, 2 * g + 8, 2 * g + 9,
                  2 * g + 16, 2 * g + 17, 2 * g + 24, 2 * g + 25]
        qcols = np.concatenate([np.arange(h * HD, (h + 1) * HD) for h in qheads])
        kvcols = np.arange(2 * g * HD, (2 * g + 2) * HD)

        wqt_np = np.ascontiguousarray(Wq[:, qcols]).astype(BF16)
        wkt_np = np.ascontiguousarray(Wk[:, kvcols]).astype(BF16)
        wv_np = np.ascontiguousarray(Wv[:, kvcols]).astype(BF16)
        wo_np = np.ascontiguousarray(Wo[qcols, :]).astype(BF16)

        in_maps.append({
            "qT": qT[b], "kT": kTt[b], "vT": vTt[b],
            "wqt": wqt_np, "wkt": wkt_np, "wv": wv_np, "wo": wo_np,
            "cosr": cos_rep, "sinr": sin_rep,
        })
    return in_maps


def kernel(q, k, v, attn_mask, Wq, Wk, Wv, Wo, bo):
    from concourse.bass_utils import run_bass_kernel_spmd

    q = np.asarray(q, dtype=np.float32)
    k = np.asarray(k, dtype=np.float32)
    v = np.asarray(v, dtype=np.float32)
    Wq = np.asarray(Wq, dtype=np.float32)
    Wk = np.asarray(Wk, dtype=np.float32)
    Wv = np.asarray(Wv, dtype=np.float32)
    Wo = np.asarray(Wo, dtype=np.float32)
    bo = np.asarray(bo, dtype=np.float32)

    if "nc" not in _CACHE:
        _CACHE["nc"] = _build_program()
    nc = _CACHE["nc"]

    in_maps = _host_inputs(q, k, v, Wq, Wk, Wv, Wo)
    trace = bool(int(os.environ.get("KERNEL_TRACE", "0")))
    res = run_bass_kernel_spmd(nc, in_maps, core_ids=list(range(NCORES)),
                               trace=trace)
    _CACHE["last_result"] = res

    out = np.zeros((B, S, D), dtype=np.float32)
    for c in range(NCORES):
        b = c // 4
        out[b] += np.asarray(res.results[c]["out"], dtype=np.float32)
    out += bo[None, None, :]
    return out


# revision 10
# speedup vs baseline: 1.7121x; 1.0362x over previous
"""GQA attention block on 8 trn2 NeuronCores.

Sharding: core c = (batch b=c//4, kv-head-pair g=c%4). Each core owns kv heads
{2g, 2g+1} and their 8 query heads (GQA tile mapping: q-head i -> kv-head i%8),
with Wq/Wk/Wv column-sharded and Wo row-sharded; host sums the 4 partial
outputs per batch and adds bo.

Device strategy (per core):
  - scores: the two heads of a pair run CONCURRENTLY as 64x128 PE row tiles
    (T0 reads SBUF partitions 0-63 = even head, T8 reads 64-127 = odd head),
    each writing its own PSUM bank of a shared [128,1024] f32 quad (a PSUM
    bank must never be written by two row tiles concurrently).
  - exp on ACT in 1024-wide chunks (amortizes the ~352-cycle ACTIVATE
    overhead), scale=1/8 folded in, bf16 out.
  - AV and all projections run as plain full-array 128x128 matmuls (single
    writer per PSUM bank). AV: lhsT = Vp chunk [128 kpos, 65] with a ones
    column giving the softmax denominator in psum row 64.
  - Q/O projection work is emitted as fine-grained "filler" units between
    attention steps so the PE stays busy while ACT chews exp.
  - RoPE: PSUM evacuated to bf16 SBUF, rotate_half via partition-shifted DVE
    copies, cos/sin combine in bf16 (fast DVE modes).
"""

import os
from contextlib import ExitStack

import numpy as np
import ml_dtypes

D = 2048
QH = 32
KVH = 8
HD = 64
B = 2
S = 2048
THETA = 1000000.0
P = 128
NCORES = 8

BF16 = ml_dtypes.bfloat16

_CACHE = {}


def _build_program():
    import concourse.bass as bass
    import concourse.tile as tile
    from concourse import bacc, mybir

    nc = bacc.Bacc(
        "TRN2",
        target_bir_lowering=False,
        debug=False,
        enable_asserts=False,
        num_devices=NCORES,
    )
    bf = mybir.dt.bfloat16
    f32 = mybir.dt.float32

    qT = nc.dram_tensor("qT", [D, S], bf, kind="ExternalInput").ap()
    kT = nc.dram_tensor("kT", [D, S], bf, kind="ExternalInput").ap()
    vT = nc.dram_tensor("vT", [D, S], bf, kind="ExternalInput").ap()
    wqt = nc.dram_tensor("wqt", [D, 512], bf, kind="ExternalInput").ap()
    wkt = nc.dram_tensor("wkt", [D, 128], bf, kind="ExternalInput").ap()
    wv = nc.dram_tensor("wv", [D, 128], bf, kind="ExternalInput").ap()
    wo = nc.dram_tensor("wo", [512, D], bf, kind="ExternalInput").ap()
    cosr = nc.dram_tensor("cosr", [P, S], bf, kind="ExternalInput").ap()
    sinr = nc.dram_tensor("sinr", [P, S], bf, kind="ExternalInput").ap()
    out = nc.dram_tensor("out", [S, D], f32, kind="ExternalOutput").ap()

    # partitioned DRAM views
    qT3 = qT.rearrange("(o p) s -> p o s", p=P)    # [128, 16, 2048]
    kT3 = kT.rearrange("(o p) s -> p o s", p=P)
    vT3 = vT.rearrange("(o p) s -> p o s", p=P)
    wqt3 = wqt.rearrange("(o p) m -> p o m", p=P)  # [128, 16, 512]
    wkt3 = wkt.rearrange("(o p) m -> p o m", p=P)  # [128, 16, 128]
    wv3 = wv.rearrange("(o p) m -> p o m", p=P)    # [128, 16, 128]
    wo3 = wo.rearrange("(o p) d -> p o d", p=P)    # [128, 4, 2048]
    out3 = out.rearrange("(t p) d -> p t d", p=P)  # [128, 16, 2048]

    scale = 1.0 / float(np.sqrt(HD))
    LO = slice(0, 64)
    HI = slice(64, 128)

    with tile.TileContext(nc) as tc, ExitStack() as ctx:
        Exp = mybir.ActivationFunctionType.Exp
        const = ctx.enter_context(tc.tile_pool(name="const", bufs=1))
        persist = ctx.enter_context(tc.tile_pool(name="persist", bufs=1))
        qpt_pool = ctx.enter_context(tc.tile_pool(name="qptp", bufs=2))
        outT_pool = ctx.enter_context(tc.tile_pool(name="outTp", bufs=2))
        vkin = ctx.enter_context(tc.tile_pool(name="vkin", bufs=2))
        qin = ctx.enter_context(tc.tile_pool(name="qin", bufs=2))
        rtmp = ctx.enter_context(tc.tile_pool(name="rtmp", bufs=2))
        fout = ctx.enter_context(tc.tile_pool(name="fout", bufs=5))
        ntmp = ctx.enter_context(tc.tile_pool(name="ntmp", bufs=2))
        etp = ctx.enter_context(tc.tile_pool(name="etp", bufs=3))
        qpsum = ctx.enter_context(tc.tile_pool(name="qpsum", bufs=2, space="PSUM"))
        apsum = ctx.enter_context(tc.tile_pool(name="apsum", bufs=2, space="PSUM"))
        ppsum = ctx.enter_context(tc.tile_pool(name="ppsum", bufs=2, space="PSUM"))

        # ---- resident weights / tables (small V/K weights first so the
        # V projection can start while the big tables stream in) ----
        wv_sb = const.tile([P, 16, 128], bf, tag="wv")
        nc.sync.dma_start(wv_sb[:], wv3[:])
        wkt_sb = const.tile([P, 16, 128], bf, tag="wkt")
        nc.sync.dma_start(wkt_sb[:], wkt3[:])
        cos_sb = const.tile([P, S], bf, tag="cos")
        nc.sync.dma_start(cos_sb[:], cosr[:])
        sin_sb = const.tile([P, S], bf, tag="sin")
        nc.sync.dma_start(sin_sb[:], sinr[:])
        wqt_sb = const.tile([P, 16, 512], bf, tag="wqt")
        nc.sync.dma_start(wqt_sb[:], wqt3[:])
        wo_sb = const.tile([P, 4, 2048], bf, tag="wo")
        nc.sync.dma_start(wo_sb[:], wo3[:])

        # ---- persistent intermediates ----
        kpt_b = persist.tile([P, S], bf, tag="kpt")       # rotated K^T pair-stacked
        vp_sb = persist.tile([P, 16, 130], bf, tag="vp")  # Vp + ones cols
        nc.vector.memset(vp_sb[:, :, 64:65], 1.0)
        nc.vector.memset(vp_sb[:, :, 129:130], 1.0)

        def rope(ps, gs, dst):
            """RoPE: ps [128,512] f32 psum (pair-stacked head dims) ->
            dst bf16 [128,512]. Evacuate early to free the bank, then bf16
            DVE ops (rotate_half = partition-shifted copies)."""
            ev = rtmp.tile([P, 512], bf, tag="ev", name="ev")
            nc.vector.tensor_copy(out=ev[:], in_=ps[:])
            rot = rtmp.tile([P, 512], bf, tag="rot", name="rot")
            for b0 in (0, 64):
                nc.vector.tensor_scalar_mul(
                    rot[b0 : b0 + 32, :], ev[b0 + 32 : b0 + 64, :], -1.0
                )
                nc.vector.tensor_copy(
                    out=rot[b0 + 32 : b0 + 64, :], in_=ev[b0 : b0 + 32, :]
                )
            t1 = rtmp.tile([P, 512], bf, tag="t1", name="t1")
            t2 = rtmp.tile([P, 512], bf, tag="t2", name="t2")
            nc.vector.tensor_mul(out=t1[:], in0=ev[:], in1=cos_sb[:, gs])
            nc.vector.tensor_mul(out=t2[:], in0=rot[:], in1=sin_sb[:, gs])
            nc.vector.tensor_add(out=dst, in0=t1[:], in1=t2[:])

        def accum(chunks, nacc, get_lhsT, get_rhs, alloc, consume,
                  prep=None, unit=2):
            """Generator emitting full-array PSUM accumulation chains, one
            chunk per bank (single writer). Yields every `unit` matmuls."""
            cnt = 0
            for desc in list(chunks):
                if prep is not None:
                    prep(desc)
                ps = alloc(desc)
                for o in range(nacc):
                    nc.tensor.matmul(
                        ps, lhsT=get_lhsT(desc, o), rhs=get_rhs(desc, o),
                        start=(o == 0), stop=(o == nacc - 1),
                    )
                    cnt += 1
                    if cnt >= unit:
                        cnt = 0
                        yield
                consume(ps, desc)

        def run_all(gen):
            for _ in gen:
                pass

        # ================= prologue: V projection =================
        # vp[s,128] per s-tile via stationary vT s-tiles, moving wv.
        def v_phase():
            vh_tiles = {}

            def prep(st):
                sc = st // 4
                if st % 4 == 0 and sc not in vh_tiles:
                    vh = vkin.tile([P, 16, 512], bf, tag="vkin", name="vh")
                    for o4 in range(0, 16, 4):
                        nc.sync.dma_start(
                            vh[:, o4 : o4 + 4, :],
                            vT3[:, o4 : o4 + 4, sc * 512 : (sc + 1) * 512],
                        )
                    vh_tiles[sc] = vh

            def alloc(st):
                return qpsum.tile([P, 1024], f32, tag="qp", name="psv")[:, :128]

            def get_lhsT(st, o):
                return vh_tiles[st // 4][:, o, (st % 4) * 128 : (st % 4 + 1) * 128]

            def get_rhs(st, o):
                return wv_sb[:, o, :]

            def consume(ps, st):
                nc.vector.tensor_copy(out=vp_sb[:, st, 0:64], in_=ps[:, 0:64])
                nc.vector.tensor_copy(out=vp_sb[:, st, 65:129], in_=ps[:, 64:128])

            return accum(range(16), 16, get_lhsT, get_rhs, alloc, consume,
                         prep=prep)

        # prefetch the first K chunk and quarter-0 Q input alongside V
        kh_prefetch = {}
        kh0 = vkin.tile([P, 16, 512], bf, tag="kh0", name="kh0", bufs=1)
        for o4 in range(0, 16, 4):
            nc.sync.dma_start(kh0[:, o4 : o4 + 4, :], kT3[:, o4 : o4 + 4, 0:512])
        kh_prefetch[0] = kh0

        # ================= prologue: K projection + RoPE =================
        def k_phase():
            kh_tiles = dict(kh_prefetch)

            def prep(ns):
                if ns in kh_tiles:
                    return
                kh = vkin.tile([P, 16, 512], bf, tag="vkin", name="kh")
                for o4 in range(0, 16, 4):
                    nc.sync.dma_start(
                        kh[:, o4 : o4 + 4, :],
                        kT3[:, o4 : o4 + 4, ns * 512 : (ns + 1) * 512],
                    )
                kh_tiles[ns] = kh

            def alloc(ns):
                return ppsum.tile([P, 512], f32, tag="pp", name="psk")

            def get_lhsT(ns, o):
                return wkt_sb[:, o, :]

            def get_rhs(ns, o):
                return kh_tiles[ns][:, o, :]

            def consume(ps, ns):
                gs = slice(ns * 512, (ns + 1) * 512)
                rope(ps, gs, kpt_b[:, gs])

            return accum(range(4), 16, get_lhsT, get_rhs, alloc, consume,
                         prep=prep)

        # ================= Q projection (one quarter) =================
        qpt_tiles = {}

        def load_qh(quarter):
            qh_sb = qin.tile([P, 16, 512], bf, tag="qin", name="qh")
            for o4 in range(0, 16, 4):
                nc.sync.dma_start(
                    qh_sb[:, o4 : o4 + 4, :],
                    qT3[:, o4 : o4 + 4, quarter * 512 : (quarter + 1) * 512],
                )
            return qh_sb

        def qproj_gen(quarter, qh_sb):
            gs = slice(quarter * 512, (quarter + 1) * 512)
            qpt_tiles[quarter] = qpt_pool.tile([P, 4, 512], bf, tag="qpt", name="qpt_q")

            def alloc(m):
                return ppsum.tile([P, 512], f32, tag="pp", name="psq")

            def get_lhsT(m, o):
                return wqt_sb[:, o, m * 128 : (m + 1) * 128]

            def get_rhs(m, o):
                return qh_sb[:, o, :]

            def consume(ps, m):
                rope(ps, gs, qpt_tiles[quarter][:, m, :])

            return accum(range(4), 16, get_lhsT, get_rhs, alloc, consume)

        # ================= output projection (one quarter) =================
        outT_tiles = {}

        def outproj_gen(quarter):
            combos = [(qi, dn) for qi in range(4) for dn in range(4)]
            outT_q = outT_tiles[quarter]

            def alloc(c):
                return ppsum.tile([P, 512], f32, tag="pp", name="psf")

            def get_lhsT(c, o):
                qi, dn = c
                return outT_q[:, o, qi * 128 : (qi + 1) * 128]

            def get_rhs(c, o):
                qi, dn = c
                return wo_sb[:, o, dn * 512 : (dn + 1) * 512]

            def consume(ps, c):
                qi, dn = c
                of = fout.tile([P, 512], f32, tag="of", name="of")
                nc.vector.tensor_copy(out=of[:], in_=ps[:])
                nc.sync.dma_start(
                    out3[:, quarter * 4 + qi, dn * 512 : (dn + 1) * 512], of[:]
                )

            return accum(combos, 4, get_lhsT, get_rhs, alloc, consume)

        # ---- prologue: V / K / Q0 round-robin so the PE computes through
        # the input DMA waits ----
        qh0 = load_qh(0)
        pro = [v_phase(), k_phase(), qproj_gen(0, qh0)]
        weights = [4, 1, 1]  # pops per round, matched to phase sizes
        while pro:
            for g, w in zip(list(pro), list(weights)):
                try:
                    for _ in range(w):
                        next(g)
                except StopIteration:
                    i = pro.index(g)
                    pro.pop(i)
                    weights.pop(i)

        # ================= main loop: flattened attention pipeline =========
        from collections import deque

        fillerq = deque()

        def pop_filler():
            while fillerq:
                try:
                    next(fillerq[0])
                    return
                except StopIteration:
                    fillerq.popleft()

        av_tiles = {}

        def av_step(q, pr, pet, pkt, last):
            first = pkt == 0
            if first:
                av0 = apsum.tile([65, 512], f32, tag="av", name="av0")
                av1 = apsum.tile([65, 512], f32, tag="av", name="av1")
                av_tiles[(q, pr)] = (av0, av1)
            av0, av1 = av_tiles[(q, pr)]
            nc.tensor.matmul(
                av0, lhsT=vp_sb[:, pkt, 0:65], rhs=pet[:, 0:512],
                start=first, stop=last,
            )
            nc.tensor.matmul(
                av1, lhsT=vp_sb[:, pkt, 65:130], rhs=pet[:, 512:1024],
                start=first, stop=last,
            )
            if last:
                outT_q = outT_tiles[q]
                for e, av in ((0, av0), (1, av1)):
                    avc = ntmp.tile([65, 512], f32, tag="avc", name="avc")
                    nc.vector.tensor_copy(out=avc[:], in_=av[:])
                    recip = ntmp.tile([1, 512], f32, tag="recip", name="recip")
                    nc.vector.reciprocal(recip[:], avc[64:65, :])
                    bc = ntmp.tile([64, 512], f32, tag="bc", name="bc")
                    nc.gpsimd.partition_broadcast(bc[:], recip[:])
                    hp = slice(e * 64, e * 64 + 64)
                    nc.vector.tensor_mul(
                        out=outT_q[hp, pr, :], in0=avc[0:64, :], in1=bc[:]
                    )

        steps = [(q, pr, kt) for q in range(4) for pr in range(4)
                 for kt in range(16)]
        prev = None
        for q, pr, kt in steps:
            if pr == 0 and kt == 0:
                outT_q = outT_pool.tile([P, 4, 512], bf, tag="outT",
                                        name="outT_q")
                outT_tiles[q] = outT_q
                if q < 3:
                    qh_next = load_qh(q + 1)
                    fillerq.append(qproj_gen(q + 1, qh_next))
            if pr == 1 and kt == 0 and q > 0:
                # deferred so outT(q-1)'s last norm (emitted during pair 0's
                # first steps) exists before any outproj unit references it
                fillerq.append(outproj_gen(q - 1))

            qpt_q = qpt_tiles[q]
            quad = qpsum.tile([P, 1024], f32, tag="qp", name="quad")
            ksl = slice(kt * 128, (kt + 1) * 128)
            nc.tensor.matmul(
                quad[:, 0:512], lhsT=kpt_b[LO, ksl],
                rhs=qpt_q[LO, pr, :], start=True, stop=True,
            )
            nc.tensor.matmul(
                quad[:, 512:1024], lhsT=kpt_b[HI, ksl],
                rhs=qpt_q[HI, pr, :], start=True, stop=True,
            )
            et = etp.tile([P, 1024], bf, tag="et", name="et")
            nc.scalar.activation(
                out=et[:], in_=quad[:], func=Exp, scale=scale
            )
            if prev is not None:
                pq, ppr, pet, pkt = prev
                av_step(pq, ppr, pet, pkt, last=(pkt == 15))
            pop_filler()
            prev = (q, pr, et, kt)
        # drain the last AV step and remaining filler
        pq, ppr, pet, pkt = prev
        av_step(pq, ppr, pet, pkt, last=True)
        while fillerq:
            try:
                next(fillerq[0])
            except StopIteration:
                fillerq.popleft()

        # epilogue: out projection of last quarter
        run_all(outproj_gen(3))

    nc.finalize()
    return nc


def _host_inputs(q, k, v, Wq, Wk, Wv, Wo):
    """Build the 8 per-core input dicts."""
    inv_freq = 1.0 / (THETA ** (np.arange(0, HD, 2, dtype=np.float32) / HD))
    t = np.arange(S, dtype=np.float32)
    freqs = np.einsum("i,j->ij", t, inv_freq)
    emb = np.concatenate([freqs, freqs], axis=-1)  # [S, 64]
    cosT = np.ascontiguousarray(np.cos(emb).T, dtype=np.float32)  # [64, S]
    sinT = np.ascontiguousarray(np.sin(emb).T, dtype=np.float32)
    cos_rep = np.concatenate([cosT, cosT], axis=0).astype(BF16)  # [128, S]
    sin_rep = np.concatenate([sinT, sinT], axis=0).astype(BF16)

    qT = [np.ascontiguousarray(q[b].T).astype(BF16) for b in range(B)]
    kTt = [np.ascontiguousarray(k[b].T).astype(BF16) for b in range(B)]
    vTt = [np.ascontiguousarray(v[b].T).astype(BF16) for b in range(B)]

    in_maps = []
    for c in range(NCORES):
        b, g = divmod(c, 4)
        # pair-interleaved: chunk i of 128 cols = (kv0 q-head i, kv1 q-head i)
        qheads = [2 * g, 2 * g + 1, 2 * g + 8, 2 * g + 9,
                  2 * g + 16, 2 * g + 17, 2 * g + 24, 2 * g + 25]
        qcols = np.concatenate([np.arange(h * HD, (h + 1) * HD) for h in qheads])
        kvcols = np.arange(2 * g * HD, (2 * g + 2) * HD)

        wqt_np = np.ascontiguousarray(Wq[:, qcols]).astype(BF16)
        wkt_np = np.ascontiguousarray(Wk[:, kvcols]).astype(BF16)
        wv_np = np.ascontiguousarray(Wv[:, kvcols]).astype(BF16)
        wo_np = np.ascontiguousarray(Wo[qcols, :]).astype(BF16)

        in_maps.append({
            "qT": qT[b], "kT": kTt[b], "vT": vTt[b],
            "wqt": wqt_np, "wkt": wkt_np, "wv": wv_np, "wo": wo_np,
            "cosr": cos_rep, "sinr": sin_rep,
        })
    return in_maps


def kernel(q, k, v, attn_mask, Wq, Wk, Wv, Wo, bo):
    from concourse.bass_utils import run_bass_kernel_spmd

    q = np.asarray(q, dtype=np.float32)
    k = np.asarray(k, dtype=np.float32)
    v = np.asarray(v, dtype=np.float32)
    Wq = np.asarray(Wq, dtype=np.float32)
    Wk = np.asarray(Wk, dtype=np.float32)
    Wv = np.asarray(Wv, dtype=np.float32)
    Wo = np.asarray(Wo, dtype=np.float32)
    bo = np.asarray(bo, dtype=np.float32)

    if "nc" not in _CACHE:
        _CACHE["nc"] = _build_program()
    nc = _CACHE["nc"]

    in_maps = _host_inputs(q, k, v, Wq, Wk, Wv, Wo)
    trace = bool(int(os.environ.get("KERNEL_TRACE", "0")))
    res = run_bass_kernel_spmd(nc, in_maps, core_ids=list(range(NCORES)),
                               trace=trace)
    _CACHE["last_result"] = res

    out = np.zeros((B, S, D), dtype=np.float32)
    for c in range(NCORES):
        b = c // 4
        out[b] += np.asarray(res.results[c]["out"], dtype=np.float32)
    out += bo[None, None, :]
    return out
